# revision 23
# baseline (speedup 1.0000x reference)
"""Trainium2 Bass kernel for nn_DeformationCorrector.

Math (per particle, F = [[a,b],[c,d]], det F > 0 for this data):
  closed-form 2x2 SVD:  y1 = (a+d)^2 + (c-b)^2,  y2 = (a-d)^2 + (c+b)^2
    sq1 = sqrt(y1), sq2 = sqrt(y2); rinv1 = 1/sq1
  polar rotation R = U@Vh = [[p,-q],[q,p]],  p = (a+d)*rinv1, q = (c-b)*rinv1
  features [sq1, sq2, a^2+c^2, b^2+d^2, ab+cd, ad-bc] @ W1eff + b1eff
  MLP 6->128->128->3 (symmetrized W3), then delta = R @ x_sym, out = delta + F.

Distribution: pure data parallel over 8 cores, contiguous shards, weights
replicated. Layout conversions (particle-major elementwise <-> feature-major
matmul) go through DRAM round trips.

Structure: variable-size blocks (small head/tail blocks for pipeline
fill/drain), stage-1/3 elementwise on GpSimd (keeping the DVE/ACT FIFOs
clear for PSUM evacuation), and a software-pipelined stage 2 where per
pipeline slot the PE runs [L1 quad(t) | L2 x4 (t-1) | L3 quad(t-2)] pinned
by an explicit same-engine dependency chain (tile-position quads run
concurrently; the four L2s share one W2 residency).
"""

from contextlib import ExitStack

import numpy as np

import concourse.bass as bass
import concourse.bacc as bacc
import concourse.tile as tile
from concourse.tile_rust import add_dep_helper
from concourse import mybir
from concourse.bass_utils import run_bass_kernel_spmd

NCORES = 8
P = 128
T = 512                        # matmul moving free dim (one PSUM bank fp32)
SB = 4                         # groups per superblock (8192 particles)
CBS = [64, 128, 320, 448, 64]  # per-block particles/partition (mult of 64)
NBLK = len(CBS)
BLKS = [P * cb for cb in CBS]
NPC = sum(BLKS)                # 131072 particles per core (padded)
NSUPS = [cb // 64 for cb in CBS]     # superblocks per block
GRPS = [4 * ns for ns in NSUPS]      # pipeline groups (2048 particles) per block
NGROUPS = sum(GRPS)            # 64
BOFF = [sum(BLKS[:b]) for b in range(NBLK)]
GOFF = [sum(GRPS[:b]) for b in range(NBLK)]
N = 1_000_000
NTOT = NCORES * NPC

FP32 = mybir.dt.float32
BF16 = mybir.dt.bfloat16
AF = mybir.ActivationFunctionType
OP = mybir.AluOpType

_built = {}
_last_results = None


def build_program():
    nc = bacc.Bacc(trn_type="TRN2")

    F_in = nc.dram_tensor("F", [NPC, 4], FP32, kind="ExternalInput")
    W1S_in = nc.dram_tensor("W1S", [P, P], BF16, kind="ExternalInput")
    W2_in = nc.dram_tensor("W2", [P, P], BF16, kind="ExternalInput")
    W3S_in = nc.dram_tensor("W3S", [P, 32], BF16, kind="ExternalInput")
    B1_in = nc.dram_tensor("B1", [P, 1], FP32, kind="ExternalInput")
    B2_in = nc.dram_tensor("B2", [P, 1], FP32, kind="ExternalInput")
    B3S_in = nc.dram_tensor("B3S", [P, 1], FP32, kind="ExternalInput")
    OUT = nc.dram_tensor("OUT", [NPC, 4], FP32, kind="ExternalOutput")

    with tile.TileContext(nc) as tc, ExitStack() as ctx:
        consts = ctx.enter_context(tc.tile_pool(name="consts", bufs=1))
        fblk = ctx.enter_context(tc.tile_pool(name="fblk", bufs=3))
        scr = ctx.enter_context(tc.tile_pool(name="scr", bufs=1))
        featp = ctx.enter_context(tc.tile_pool(name="featp", bufs=2))
        dramp = ctx.enter_context(tc.tile_pool(name="dramp", bufs=NBLK, space="DRAM"))
        fmp = ctx.enter_context(tc.tile_pool(name="fmp", bufs=2))
        hp = ctx.enter_context(tc.tile_pool(name="hp", bufs=4))
        xp = ctx.enter_context(tc.tile_pool(name="xp", bufs=2))
        outp = ctx.enter_context(tc.tile_pool(name="outp", bufs=2))
        psz1 = ctx.enter_context(tc.tile_pool(name="psz1", bufs=2, space="PSUM"))
        psz2 = ctx.enter_context(tc.tile_pool(name="psz2", bufs=3, space="PSUM"))
        psx = ctx.enter_context(tc.tile_pool(name="psx", bufs=1, space="PSUM"))

        # ---- constants ----
        w1s_sb = consts.tile([P, P], BF16)
        nc.sync.dma_start(out=w1s_sb[:], in_=W1S_in[:, :])
        w2_sb = consts.tile([P, P], BF16)
        nc.sync.dma_start(out=w2_sb[:], in_=W2_in[:, :])
        w3s_sb = consts.tile([P, 32], BF16)
        nc.sync.dma_start(out=w3s_sb[:], in_=W3S_in[:, :])
        b1_sb = consts.tile([P, 1], FP32)
        nc.sync.dma_start(out=b1_sb[:], in_=B1_in[:, :])
        b2_sb = consts.tile([P, 1], FP32)
        nc.sync.dma_start(out=b2_sb[:], in_=B2_in[:, :])
        b3s_sb = consts.tile([P, 1], FP32)
        nc.sync.dma_start(out=b3s_sb[:], in_=B3S_in[:, :])

        f_tiles = [None] * NBLK
        pq_tiles = [None] * NBLK
        featd_tiles = [None] * NBLK
        xd_tiles = [None] * NBLK
        s1state = [None] * NBLK

        # ============ stage 1: particle-major features ============
        # Emitted in three pieces so no engine FIFO head-of-line-blocks on a
        # cross-engine chain: front (gpsimd, or DVE for the head block),
        # then sqrt (ACT), then rinv(DVE)+pq(gpsimd)+featd-store.
        def stage1_front(b):
            head = b == 0
            E = nc.vector if head else nc.gpsimd
            G = nc.gpsimd
            CB = CBS[b]

            f_sb = fblk.tile([P, 4 * CB], FP32, tag="F", name=f"f_sb{b}",
                             padded_shape=[P, 4 * max(CBS)])
            F_bv = F_in[BOFF[b] : BOFF[b] + BLKS[b], :].rearrange(
                "(i g j) k -> i g (j k)", i=32, g=4
            )
            for g in range(4):
                nc.sync.dma_start(out=f_sb[32 * g : 32 * g + 32, :], in_=F_bv[:, g, :])
            f_tiles[b] = f_sb
            fr = f_sb.rearrange("p (c k) -> p c k", k=4)
            fr2 = f_sb.rearrange("p (c s k) -> p c s k", s=2, k=2)

            # feature rows: [sq1, sq2, f2=a2+c2, f4=b2+d2, f3=ab+cd, f5=ad-bc]
            feat_sb = featp.tile([P, 6 * CB], BF16, tag="feat", name=f"feat_sb{b}",
                                 padded_shape=[P, 6 * max(CBS)])
            fv = feat_sb.rearrange("p (f c) -> p f c", f=6)

            sq_sb = scr.tile([P, 4 * CB], FP32, tag="sq", name=f"sq_sb{b}",
                             padded_shape=[P, 4 * max(CBS)])
            sqr = sq_sb.rearrange("p (c u k) -> p c u k", u=2, k=2)
            pp_sb = scr.tile([P, 2 * CB], FP32, tag="pp", name=f"pp_sb{b}",
                             padded_shape=[P, 2 * max(CBS)])
            ppv = pp_sb.rearrange("p (c s) -> p c s", s=2)
            ad_sb = scr.tile([P, CB], FP32, tag="ad", name=f"ad_sb{b}",
                             padded_shape=[P, max(CBS)])
            bc_sb = scr.tile([P, CB], FP32, tag="bc", name=f"bc_sb{b}",
                             padded_shape=[P, max(CBS)])
            # sv4 = [s=a+d | v=c-b | d2=a-d | v2=c+b]
            sv4_sb = scr.tile([P, 4 * CB], FP32, tag="sv4", name=f"sv4_sb{b}",
                              padded_shape=[P, 4 * max(CBS)])
            sv4v = sv4_sb.rearrange("p (e c) -> p e c", e=4)
            s4_sb = scr.tile([P, 4 * CB], FP32, tag="s4", name=f"s4_sb{b}",
                             padded_shape=[P, 4 * max(CBS)])
            y12_sb = scr.tile([P, 2 * CB], FP32, tag="y12", name=f"y12_sb{b}",
                              padded_shape=[P, 2 * max(CBS)])
            rinv_sb = scr.tile([P, CB], FP32, tag="rinv", name=f"rinv_sb{b}",
                               padded_shape=[P, max(CBS)])
            pq_sb = fblk.tile([P, 2 * CB], FP32, tag="pq", name=f"pq_sb{b}",
                              padded_shape=[P, 2 * max(CBS)])
            pq_tiles[b] = pq_sb

            # squares of all 4 components (contiguous)
            E.tensor_tensor(out=sq_sb[:], in0=f_sb[:], in1=f_sb[:], op=OP.mult)
            # pp = [a*b, c*d] ; f3 = ab + cd
            G.tensor_tensor(
                out=ppv[:, :, :], in0=fr2[:, :, :, 0], in1=fr2[:, :, :, 1], op=OP.mult
            )
            G.tensor_tensor(out=fv[:, 4, :], in0=ppv[:, :, 0], in1=ppv[:, :, 1], op=OP.add)
            # f5 = ad - bc
            G.tensor_tensor(out=ad_sb[:], in0=fr[:, :, 0], in1=fr[:, :, 3], op=OP.mult)
            G.tensor_tensor(out=bc_sb[:], in0=fr[:, :, 1], in1=fr[:, :, 2], op=OP.mult)
            G.tensor_tensor(out=fv[:, 5, :], in0=ad_sb[:], in1=bc_sb[:], op=OP.subtract)
            # [f2|f4] = [aa|bb] + [cc|dd]
            E.tensor_tensor(
                out=feat_sb[:, 2 * CB : 4 * CB].rearrange("p (s c) -> p s c", s=2),
                in0=sqr[:, :, 0, :].rearrange("p c k -> p k c"),
                in1=sqr[:, :, 1, :].rearrange("p c k -> p k c"),
                op=OP.add,
            )
            # sv4
            E.tensor_tensor(out=sv4v[:, 0, :], in0=fr[:, :, 0], in1=fr[:, :, 3], op=OP.add)
            E.tensor_tensor(out=sv4v[:, 1, :], in0=fr[:, :, 2], in1=fr[:, :, 1], op=OP.subtract)
            E.tensor_tensor(out=sv4v[:, 2, :], in0=fr[:, :, 0], in1=fr[:, :, 3], op=OP.subtract)
            E.tensor_tensor(out=sv4v[:, 3, :], in0=fr[:, :, 2], in1=fr[:, :, 1], op=OP.add)
            E.tensor_tensor(out=s4_sb[:], in0=sv4_sb[:], in1=sv4_sb[:], op=OP.mult)
            # y1 = s^2+v^2, y2 = d2^2+v2^2  (both nonnegative by construction)
            s4j = s4_sb.rearrange("p (j k c) -> p j k c", j=2, k=2)
            E.tensor_tensor(
                out=y12_sb.rearrange("p (j c) -> p j c", j=2),
                in0=s4j[:, :, 0, :], in1=s4j[:, :, 1, :], op=OP.add,
            )
            s1state[b] = {
                "feat": feat_sb, "y12": y12_sb, "rinv": rinv_sb, "sv4": sv4_sb,
            }

        def stage1_sqrt(b):
            st = s1state[b]
            CB = CBS[b]
            # [sq1|sq2] = sqrt(y12)  -> feature rows 0,1
            nc.scalar.activation(
                out=st["feat"][:, 0 : 2 * CB], in_=st["y12"][:], func=AF.Sqrt)

        def stage1_finish(b):
            head = b == 0
            G = nc.vector if head else nc.gpsimd
            st = s1state[b]
            CB = CBS[b]
            feat_sb = st["feat"]
            # rinv1 = 1/sq1 computed as (1/y1) * sq1 (reciprocal needs fp32)
            nc.vector.reciprocal_approx_fast(
                out=st["rinv"][:], in_=st["y12"][:, 0:CB])
            G.tensor_tensor(
                out=st["rinv"][:], in0=st["rinv"][:], in1=feat_sb[:, 0:CB],
                op=OP.mult,
            )
            pq_sb = pq_tiles[b]
            G.tensor_tensor(
                out=pq_sb.rearrange("p (s c) -> p s c", s=2),
                in0=st["sv4"].rearrange("p (e c) -> p e c", e=4)[:, 0:2, :],
                in1=st["rinv"][:].unsqueeze(1).to_broadcast([P, 2, CB]),
                op=OP.mult,
            )
            featd = dramp.tile([24, BLKS[b] // 4], BF16, tag=f"featd{b}",
                               name=f"featd{b}", bufs=1)
            featd_tiles[b] = featd
            for g in range(4):
                nc.sync.dma_start(
                    out=featd[6 * g : 6 * g + 6, :].rearrange("f (i j) -> i f j", j=CB),
                    in_=feat_sb[32 * g : 32 * g + 32, :].rearrange("i (f j) -> i f j", j=CB),
                )
            xd = dramp.tile([12, BLKS[b] // 4], FP32, tag=f"xd{b}",
                            name=f"xd{b}", bufs=1)
            xd_tiles[b] = xd

        # ============ stage 2: feature-major MLP, software-pipelined ============
        last_mm = [None]

        def mm(*args, **kwargs):
            inst = nc.tensor.matmul(*args, **kwargs).ins
            if last_mm[0] is not None:
                add_dep_helper(inst, last_mm[0], reason="pe-order")
            last_mm[0] = inst
            return inst

        supers = {}
        gstate = {}

        def gidx(i):
            b = 0
            while i >= GOFF[b] + GRPS[b]:
                b += 1
            r = i - GOFF[b]
            return b, r // SB, r % SB

        def ensure_super(b, s):
            if (b, s) in supers:
                return supers[(b, s)]
            featd = featd_tiles[b]
            featfm = fmp.tile([P, SB * T], BF16, tag="ffm", name=f"ffm{b}_{s}")
            for g in range(4):
                nc.sync.dma_start(
                    out=featfm[32 * g : 32 * g + 6, :],
                    in_=featd[6 * g : 6 * g + 6, SB * T * s : SB * T * (s + 1)],
                )
            x_sb = xp.tile([P, SB * T], FP32, tag="xsb", name=f"xsb{b}_{s}")
            sup = {"ffm_gv": featfm.rearrange("(g r) c -> g r c", g=4), "x_sb": x_sb}
            supers[(b, s)] = sup
            return sup

        def phase_a(i):
            b, s, i2 = gidx(i)
            sup = ensure_super(b, s)
            z1p = [
                psz1.tile([P, 2 * T], FP32, tag="z1", name=f"z1_{i}_0"),
                psz1.tile([P, 2 * T], FP32, tag="z1", name=f"z1_{i}_1"),
            ]
            for g in range(4):
                mm(
                    out=z1p[g // 2][:, (g % 2) * T : (g % 2 + 1) * T],
                    lhsT=w1s_sb[32 * g : 32 * g + 6, :],
                    rhs=sup["ffm_gv"][g, :6, i2 * T : (i2 + 1) * T],
                    tile_position=(32 * g, 0),
                )
            h1p = [
                hp.tile([P, 2 * T], BF16, tag="h1", name=f"h1_{i}_0"),
                hp.tile([P, 2 * T], BF16, tag="h1", name=f"h1_{i}_1"),
            ]
            nc.scalar.activation(
                out=h1p[0][:], in_=z1p[0][:], func=AF.Relu, bias=b1_sb[:]
            )
            nc.vector.tensor_scalar(
                out=h1p[1][:], in0=z1p[1][:], scalar1=b1_sb[:],
                scalar2=0.0, op0=OP.add, op1=OP.max,
            )
            gstate[i] = {"h1p": h1p}
            if i2 == SB - 1 and i + 1 < NGROUPS:
                nb, ns, _ = gidx(i + 1)
                if featd_tiles[nb] is not None:
                    ensure_super(nb, ns)

        def phase_b(i):
            st = gstate[i]
            h1p = st["h1p"]
            z2s = [
                psz2.tile([P, T], FP32, tag="z2", name=f"z2_{i}_{g}")
                for g in range(4)
            ]
            for g in range(4):
                mm(
                    out=z2s[g][:],
                    lhsT=w2_sb[:],
                    rhs=h1p[g // 2][:, (g % 2) * T : (g % 2 + 1) * T],
                )
            h2s = []
            for g in range(4):
                h2 = hp.tile([P, T], BF16, tag="h2", name=f"h2_{i}_{g}", bufs=8)
                h2s.append(h2)
                if g % 2 == 0:
                    nc.scalar.activation(
                        out=h2[:], in_=z2s[g][:], func=AF.Relu, bias=b2_sb[:]
                    )
                else:
                    nc.vector.tensor_scalar(
                        out=h2[:], in0=z2s[g][:], scalar1=b2_sb[:],
                        scalar2=0.0, op0=OP.add, op1=OP.max,
                    )
            st["h2s"] = h2s

        def phase_c(i):
            b, s, i2 = gidx(i)
            st = gstate.pop(i)
            sup = supers[(b, s)]
            x_ps = psx.tile([P, T], FP32, tag="x", name=f"xps{i}")
            for g in range(4):
                mm(
                    out=x_ps[32 * g : 32 * g + 32, :],
                    lhsT=w3s_sb[:, :],
                    rhs=st["h2s"][g][:],
                    tile_position=(0, 32 * g),
                )
            nc.scalar.activation(
                out=sup["x_sb"][:, i2 * T : (i2 + 1) * T], in_=x_ps[:],
                func=AF.Identity, bias=b3s_sb[:],
            )
            if i2 == SB - 1:
                xd = xd_tiles[b]
                for g in range(4):
                    nc.sync.dma_start(
                        out=xd[3 * g : 3 * g + 3, SB * T * s : SB * T * (s + 1)],
                        in_=sup["x_sb"][32 * g : 32 * g + 3, :],
                    )
                del supers[(b, s)]

        # ============ stage 3: particle-major backend ============
        def stage3(b):
            tail = b == NBLK - 1
            G = nc.gpsimd
            E = nc.vector if tail else nc.gpsimd
            CB = CBS[b]
            xd = xd_tiles[b]
            f_sb = f_tiles[b]
            pqv = pq_tiles[b].rearrange("p (s c) -> p s c", s=2)
            fr = f_sb.rearrange("p (c k) -> p c k", k=4)

            xs_all = xp.tile([P, 3 * CB], FP32, tag="xsall", name=f"xsall{b}",
                             padded_shape=[P, 3 * max(CBS)])
            for g in range(4):
                nc.sync.dma_start(
                    out=xs_all[32 * g : 32 * g + 32, :].rearrange("i (k j) -> i k j", j=CB),
                    in_=xd[3 * g : 3 * g + 3, :].rearrange("k (i j) -> i k j", j=CB),
                )
            pall = scr.tile([P, 3 * CB], FP32, tag="pall", name=f"pall{b}",
                            padded_shape=[P, 3 * max(CBS)])
            pall_v = pall.rearrange("p (k c) -> p k c", k=3)
            G.tensor_tensor(
                out=pall[:], in0=xs_all[:],
                in1=pqv[:, 0, :].unsqueeze(1).to_broadcast([P, 3, CB]), op=OP.mult,
            )
            qall = scr.tile([P, 3 * CB], FP32, tag="qall", name=f"qall{b}",
                            padded_shape=[P, 3 * max(CBS)])
            qall_v = qall.rearrange("p (k c) -> p k c", k=3)
            E.tensor_tensor(
                out=qall[:], in0=xs_all[:],
                in1=pqv[:, 1, :].unsqueeze(1).to_broadcast([P, 3, CB]), op=OP.mult,
            )
            out_sb = outp.tile([P, 4 * CB], FP32, tag="out", name=f"out_sb{b}",
                               padded_shape=[P, 4 * max(CBS)])
            ov = out_sb.rearrange("p (c k) -> p c k", k=4)
            t0 = scr.tile([P, CB], FP32, tag="t0", name=f"t0_{b}",
                          padded_shape=[P, max(CBS)])
            G.tensor_tensor(out=t0[:], in0=pall_v[:, 0], in1=qall_v[:, 1], op=OP.subtract)
            G.tensor_tensor(out=ov[:, :, 0], in0=t0[:], in1=fr[:, :, 0], op=OP.add)
            t1 = scr.tile([P, CB], FP32, tag="t1", name=f"t1_{b}",
                          padded_shape=[P, max(CBS)])
            E.tensor_tensor(out=t1[:], in0=pall_v[:, 1], in1=qall_v[:, 2], op=OP.subtract)
            E.tensor_tensor(out=ov[:, :, 1], in0=t1[:], in1=fr[:, :, 1], op=OP.add)
            t2 = scr.tile([P, CB], FP32, tag="t2", name=f"t2_{b}",
                          padded_shape=[P, max(CBS)])
            G.tensor_tensor(out=t2[:], in0=qall_v[:, 0], in1=pall_v[:, 1], op=OP.add)
            G.tensor_tensor(out=ov[:, :, 2], in0=t2[:], in1=fr[:, :, 2], op=OP.add)
            t3 = scr.tile([P, CB], FP32, tag="t3", name=f"t3_{b}",
                          padded_shape=[P, max(CBS)])
            E.tensor_tensor(out=t3[:], in0=qall_v[:, 1], in1=pall_v[:, 2], op=OP.add)
            E.tensor_tensor(out=ov[:, :, 3], in0=t3[:], in1=fr[:, :, 3], op=OP.add)

            OUT_bv = OUT[BOFF[b] : BOFF[b] + BLKS[b], :].rearrange(
                "(i g j) k -> i g (j k)", i=32, g=4
            )
            for g in range(4):
                nc.sync.dma_start(out=OUT_bv[:, g, :], in_=out_sb[32 * g : 32 * g + 32, :])

        # ============ emission order (pipelined) ============
        # stage-1 pieces are spread over iterations so cross-engine waits
        # never sit at the head of a busy FIFO.
        front_at, sqrt_at, fin_at = {}, {2: 1}, {3: 1}
        for b in range(2, NBLK):
            ft = GOFF[b] - GRPS[b - 1]
            front_at[ft] = b
            sqrt_at[ft + 4] = b
            fin_at[ft + 6] = b

        stage1_front(0)
        stage1_sqrt(0)
        stage1_finish(0)
        stage1_front(1)
        for t in range(NGROUPS + 2):
            if 0 <= t - 1 < NGROUPS:
                phase_b(t - 1)
            if t < NGROUPS:
                phase_a(t)
                if t in front_at:
                    stage1_front(front_at[t])
            if t in sqrt_at:
                stage1_sqrt(sqrt_at[t])
            if t in fin_at:
                stage1_finish(fin_at[t])
            if 0 <= t - 2 < NGROUPS:
                phase_c(t - 2)
                b, s, i2 = gidx(t - 2)
                if s == NSUPS[b] - 1 and i2 == SB - 1:
                    stage3(b)

    nc.finalize()
    return nc


def prep_weights(W1, b1, W2, b2, W3, b3):
    """Host-side weight transforms (tiny)."""
    W1 = np.asarray(W1, np.float32)
    b1 = np.asarray(b1, np.float32)
    W2 = np.asarray(W2, np.float32)
    b2 = np.asarray(b2, np.float32)
    W3 = np.asarray(W3, np.float32)
    b3 = np.asarray(b3, np.float32)
    # features: [sq1, sq2, a2+c2, b2+d2, ab+cd, ad-bc]
    W1eff = np.stack(
        [
            0.5 * (W1[0] + W1[1]),
            0.5 * (W1[0] - W1[1]),
            W1[2],
            W1[5],
            W1[3] + W1[4],
            W1[6],
        ],
        axis=0,
    )  # [6, 128]
    b1eff = b1 - (W1[0] + W1[1] + W1[2] + W1[5] + W1[6])
    W1S = np.zeros((P, P), np.float32)
    for g in range(4):
        W1S[32 * g : 32 * g + 6, :] = W1eff
    # symmetrized third layer: x_sym = [x00, (x01+x10)/2, x11]
    W3S = np.zeros((P, 32), np.float32)
    W3S[:, 0] = W3[:, 0]
    W3S[:, 1] = 0.5 * (W3[:, 1] + W3[:, 2])
    W3S[:, 2] = W3[:, 3]
    b3S3 = np.array([b3[0], 0.5 * (b3[1] + b3[2]), b3[3]], np.float32)
    B3S = np.zeros((P, 1), np.float32)
    for j in range(4):
        B3S[32 * j : 32 * j + 3, 0] = b3S3
    import ml_dtypes
    return {
        "W1S": W1S.astype(ml_dtypes.bfloat16),
        "W2": W2.astype(ml_dtypes.bfloat16),
        "W3S": W3S.astype(ml_dtypes.bfloat16),
        "B1": b1eff.reshape(P, 1).astype(np.float32),
        "B2": b2.reshape(P, 1).astype(np.float32),
        "B3S": B3S,
    }


def kernel(F, W1, b1, W2, b2, W3, b3):
    global _last_results
    F = np.asarray(F, np.float32).reshape(-1, 4)
    n = F.shape[0]
    assert n == N, f"expected {N} particles, got {n}"

    if "nc" not in _built:
        _built["nc"] = build_program()
    nc = _built["nc"]

    wmaps = prep_weights(W1, b1, W2, b2, W3, b3)
    Fpad = np.empty((NTOT, 4), np.float32)
    Fpad[:n] = F
    Fpad[n:] = np.array([1.0, 0.1, 0.0, 1.0], np.float32)

    in_maps = []
    for i in range(NCORES):
        m = {"F": np.ascontiguousarray(Fpad[i * NPC : (i + 1) * NPC])}
        m.update(wmaps)
        in_maps.append(m)

    res = run_bass_kernel_spmd(nc, in_maps, core_ids=list(range(NCORES)))
    _last_results = res
    out = np.concatenate([r["OUT"] for r in res.results], axis=0)[:n]
    return out.reshape(n, 2, 2).astype(np.float32)


# revision 24
# speedup vs baseline: 1.0113x; 1.0113x over previous
"""Trainium2 Bass kernel for nn_DeformationCorrector.

Math (per particle, F = [[a,b],[c,d]], det F > 0 for this data):
  closed-form 2x2 SVD:  y1 = (a+d)^2 + (c-b)^2,  y2 = (a-d)^2 + (c+b)^2
    sq1 = sqrt(y1), sq2 = sqrt(y2); rinv1 = 1/sq1
  polar rotation R = U@Vh = [[p,-q],[q,p]],  p = (a+d)*rinv1, q = (c-b)*rinv1
  features [sq1, sq2, a^2+c^2, b^2+d^2, ab+cd, ad-bc] @ W1eff + b1eff
  MLP 6->128->128->3 (symmetrized W3), then delta = R @ x_sym, out = delta + F.

Distribution: pure data parallel over 8 cores, contiguous shards, weights
replicated. Layout conversions (particle-major elementwise <-> feature-major
matmul) go through DRAM round trips.

Structure: variable-size blocks (small head/tail blocks for pipeline
fill/drain), stage-1/3 elementwise on GpSimd (keeping the DVE/ACT FIFOs
clear for PSUM evacuation), and a software-pipelined stage 2 where per
pipeline slot the PE runs [L1 quad(t) | L2 x4 (t-1) | L3 quad(t-2)] pinned
by an explicit same-engine dependency chain (tile-position quads run
concurrently; the four L2s share one W2 residency).
"""

from contextlib import ExitStack

import numpy as np

import concourse.bass as bass
import concourse.bacc as bacc
import concourse.tile as tile
from concourse.tile_rust import add_dep_helper
from concourse import mybir
from concourse.bass_utils import run_bass_kernel_spmd

NCORES = 8
P = 128
T = 512                        # matmul moving free dim (one PSUM bank fp32)
SB = 4                         # groups per superblock (8192 particles)
CBS = [64, 192, 448, 256, 64]  # per-block particles/partition (mult of 64)
NBLK = len(CBS)
BLKS = [P * cb for cb in CBS]
NPC = sum(BLKS)                # 131072 particles per core (padded)
NSUPS = [cb // 64 for cb in CBS]     # superblocks per block
GRPS = [4 * ns for ns in NSUPS]      # pipeline groups (2048 particles) per block
NGROUPS = sum(GRPS)            # 64
BOFF = [sum(BLKS[:b]) for b in range(NBLK)]
GOFF = [sum(GRPS[:b]) for b in range(NBLK)]
N = 1_000_000
NTOT = NCORES * NPC

FP32 = mybir.dt.float32
BF16 = mybir.dt.bfloat16
AF = mybir.ActivationFunctionType
OP = mybir.AluOpType

_built = {}
_last_results = None


def build_program():
    nc = bacc.Bacc(trn_type="TRN2")

    F_in = nc.dram_tensor("F", [NPC, 4], FP32, kind="ExternalInput")
    W1S_in = nc.dram_tensor("W1S", [P, P], BF16, kind="ExternalInput")
    W2_in = nc.dram_tensor("W2", [P, P], BF16, kind="ExternalInput")
    W3S_in = nc.dram_tensor("W3S", [P, 32], BF16, kind="ExternalInput")
    B1_in = nc.dram_tensor("B1", [P, 1], FP32, kind="ExternalInput")
    B2_in = nc.dram_tensor("B2", [P, 1], FP32, kind="ExternalInput")
    B3S_in = nc.dram_tensor("B3S", [P, 1], FP32, kind="ExternalInput")
    OUT = nc.dram_tensor("OUT", [NPC, 4], FP32, kind="ExternalOutput")

    with tile.TileContext(nc) as tc, ExitStack() as ctx:
        consts = ctx.enter_context(tc.tile_pool(name="consts", bufs=1))
        fblk = ctx.enter_context(tc.tile_pool(name="fblk", bufs=3))
        scr = ctx.enter_context(tc.tile_pool(name="scr", bufs=1))
        featp = ctx.enter_context(tc.tile_pool(name="featp", bufs=2))
        dramp = ctx.enter_context(tc.tile_pool(name="dramp", bufs=NBLK, space="DRAM"))
        fmp = ctx.enter_context(tc.tile_pool(name="fmp", bufs=2))
        hp = ctx.enter_context(tc.tile_pool(name="hp", bufs=4))
        xp = ctx.enter_context(tc.tile_pool(name="xp", bufs=2))
        outp = ctx.enter_context(tc.tile_pool(name="outp", bufs=2))
        psz1 = ctx.enter_context(tc.tile_pool(name="psz1", bufs=2, space="PSUM"))
        psz2 = ctx.enter_context(tc.tile_pool(name="psz2", bufs=3, space="PSUM"))
        psx = ctx.enter_context(tc.tile_pool(name="psx", bufs=1, space="PSUM"))

        # ---- constants ----
        w1s_sb = consts.tile([P, P], BF16)
        nc.sync.dma_start(out=w1s_sb[:], in_=W1S_in[:, :])
        w2_sb = consts.tile([P, P], BF16)
        nc.sync.dma_start(out=w2_sb[:], in_=W2_in[:, :])
        w3s_sb = consts.tile([P, 32], BF16)
        nc.sync.dma_start(out=w3s_sb[:], in_=W3S_in[:, :])
        b1_sb = consts.tile([P, 1], FP32)
        nc.sync.dma_start(out=b1_sb[:], in_=B1_in[:, :])
        b2_sb = consts.tile([P, 1], FP32)
        nc.sync.dma_start(out=b2_sb[:], in_=B2_in[:, :])
        b3s_sb = consts.tile([P, 1], FP32)
        nc.sync.dma_start(out=b3s_sb[:], in_=B3S_in[:, :])

        f_tiles = [None] * NBLK
        pq_tiles = [None] * NBLK
        featd_tiles = [None] * NBLK
        xd_tiles = [None] * NBLK
        s1state = [None] * NBLK

        # ============ stage 1: particle-major features ============
        # Emitted in three pieces so no engine FIFO head-of-line-blocks on a
        # cross-engine chain: front (gpsimd, or DVE for the head block),
        # then sqrt (ACT), then rinv(DVE)+pq(gpsimd)+featd-store.
        def stage1_front(b):
            head = b == 0
            E = nc.vector if head else nc.gpsimd
            G = nc.gpsimd
            CB = CBS[b]

            f_sb = fblk.tile([P, 4 * CB], FP32, tag="F", name=f"f_sb{b}",
                             padded_shape=[P, 4 * max(CBS)])
            F_bv = F_in[BOFF[b] : BOFF[b] + BLKS[b], :].rearrange(
                "(i g j) k -> i g (j k)", i=32, g=4
            )
            for g in range(4):
                nc.sync.dma_start(out=f_sb[32 * g : 32 * g + 32, :], in_=F_bv[:, g, :])
            f_tiles[b] = f_sb
            fr = f_sb.rearrange("p (c k) -> p c k", k=4)
            fr2 = f_sb.rearrange("p (c s k) -> p c s k", s=2, k=2)

            # feature rows: [sq1, sq2, f2=a2+c2, f4=b2+d2, f3=ab+cd, f5=ad-bc]
            feat_sb = featp.tile([P, 6 * CB], BF16, tag="feat", name=f"feat_sb{b}",
                                 padded_shape=[P, 6 * max(CBS)])
            fv = feat_sb.rearrange("p (f c) -> p f c", f=6)

            sq_sb = scr.tile([P, 4 * CB], FP32, tag="sq", name=f"sq_sb{b}",
                             padded_shape=[P, 4 * max(CBS)])
            sqr = sq_sb.rearrange("p (c u k) -> p c u k", u=2, k=2)
            pp_sb = scr.tile([P, 2 * CB], FP32, tag="pp", name=f"pp_sb{b}",
                             padded_shape=[P, 2 * max(CBS)])
            ppv = pp_sb.rearrange("p (c s) -> p c s", s=2)
            ad_sb = scr.tile([P, CB], FP32, tag="ad", name=f"ad_sb{b}",
                             padded_shape=[P, max(CBS)])
            bc_sb = scr.tile([P, CB], FP32, tag="bc", name=f"bc_sb{b}",
                             padded_shape=[P, max(CBS)])
            # sv4 = [s=a+d | v=c-b | d2=a-d | v2=c+b]
            sv4_sb = scr.tile([P, 4 * CB], FP32, tag="sv4", name=f"sv4_sb{b}",
                              padded_shape=[P, 4 * max(CBS)])
            sv4v = sv4_sb.rearrange("p (e c) -> p e c", e=4)
            s4_sb = scr.tile([P, 4 * CB], FP32, tag="s4", name=f"s4_sb{b}",
                             padded_shape=[P, 4 * max(CBS)])
            y12_sb = scr.tile([P, 2 * CB], FP32, tag="y12", name=f"y12_sb{b}",
                              padded_shape=[P, 2 * max(CBS)])
            rinv_sb = scr.tile([P, CB], FP32, tag="rinv", name=f"rinv_sb{b}",
                               padded_shape=[P, max(CBS)])
            pq_sb = fblk.tile([P, 2 * CB], FP32, tag="pq", name=f"pq_sb{b}",
                              padded_shape=[P, 2 * max(CBS)])
            pq_tiles[b] = pq_sb

            # squares of all 4 components (contiguous)
            E.tensor_tensor(out=sq_sb[:], in0=f_sb[:], in1=f_sb[:], op=OP.mult)
            # pp = [a*b, c*d] ; f3 = ab + cd
            G.tensor_tensor(
                out=ppv[:, :, :], in0=fr2[:, :, :, 0], in1=fr2[:, :, :, 1], op=OP.mult
            )
            G.tensor_tensor(out=fv[:, 4, :], in0=ppv[:, :, 0], in1=ppv[:, :, 1], op=OP.add)
            # f5 = ad - bc
            G.tensor_tensor(out=ad_sb[:], in0=fr[:, :, 0], in1=fr[:, :, 3], op=OP.mult)
            G.tensor_tensor(out=bc_sb[:], in0=fr[:, :, 1], in1=fr[:, :, 2], op=OP.mult)
            G.tensor_tensor(out=fv[:, 5, :], in0=ad_sb[:], in1=bc_sb[:], op=OP.subtract)
            # [f2|f4] = [aa|bb] + [cc|dd]
            E.tensor_tensor(
                out=feat_sb[:, 2 * CB : 4 * CB].rearrange("p (s c) -> p s c", s=2),
                in0=sqr[:, :, 0, :].rearrange("p c k -> p k c"),
                in1=sqr[:, :, 1, :].rearrange("p c k -> p k c"),
                op=OP.add,
            )
            # sv4
            E.tensor_tensor(out=sv4v[:, 0, :], in0=fr[:, :, 0], in1=fr[:, :, 3], op=OP.add)
            E.tensor_tensor(out=sv4v[:, 1, :], in0=fr[:, :, 2], in1=fr[:, :, 1], op=OP.subtract)
            E.tensor_tensor(out=sv4v[:, 2, :], in0=fr[:, :, 0], in1=fr[:, :, 3], op=OP.subtract)
            E.tensor_tensor(out=sv4v[:, 3, :], in0=fr[:, :, 2], in1=fr[:, :, 1], op=OP.add)
            E.tensor_tensor(out=s4_sb[:], in0=sv4_sb[:], in1=sv4_sb[:], op=OP.mult)
            # y1 = s^2+v^2, y2 = d2^2+v2^2  (both nonnegative by construction)
            s4j = s4_sb.rearrange("p (j k c) -> p j k c", j=2, k=2)
            E.tensor_tensor(
                out=y12_sb.rearrange("p (j c) -> p j c", j=2),
                in0=s4j[:, :, 0, :], in1=s4j[:, :, 1, :], op=OP.add,
            )
            s1state[b] = {
                "feat": feat_sb, "y12": y12_sb, "rinv": rinv_sb, "sv4": sv4_sb,
            }

        def stage1_sqrt(b):
            st = s1state[b]
            CB = CBS[b]
            # [sq1|sq2] = sqrt(y12)  -> feature rows 0,1
            nc.scalar.activation(
                out=st["feat"][:, 0 : 2 * CB], in_=st["y12"][:], func=AF.Sqrt)

        def stage1_finish(b):
            head = b == 0
            G = nc.vector if head else nc.gpsimd
            st = s1state[b]
            CB = CBS[b]
            feat_sb = st["feat"]
            # rinv1 = 1/sq1 computed as (1/y1) * sq1 (reciprocal needs fp32)
            nc.vector.reciprocal_approx_fast(
                out=st["rinv"][:], in_=st["y12"][:, 0:CB])
            G.tensor_tensor(
                out=st["rinv"][:], in0=st["rinv"][:], in1=feat_sb[:, 0:CB],
                op=OP.mult,
            )
            pq_sb = pq_tiles[b]
            G.tensor_tensor(
                out=pq_sb.rearrange("p (s c) -> p s c", s=2),
                in0=st["sv4"].rearrange("p (e c) -> p e c", e=4)[:, 0:2, :],
                in1=st["rinv"][:].unsqueeze(1).to_broadcast([P, 2, CB]),
                op=OP.mult,
            )
            featd = dramp.tile([24, BLKS[b] // 4], BF16, tag=f"featd{b}",
                               name=f"featd{b}", bufs=1)
            featd_tiles[b] = featd
            for g in range(4):
                nc.sync.dma_start(
                    out=featd[6 * g : 6 * g + 6, :].rearrange("f (i j) -> i f j", j=CB),
                    in_=feat_sb[32 * g : 32 * g + 32, :].rearrange("i (f j) -> i f j", j=CB),
                )
            xd = dramp.tile([12, BLKS[b] // 4], FP32, tag=f"xd{b}",
                            name=f"xd{b}", bufs=1)
            xd_tiles[b] = xd

        # ============ stage 2: feature-major MLP, software-pipelined ============
        last_mm = [None]

        def mm(*args, **kwargs):
            inst = nc.tensor.matmul(*args, **kwargs).ins
            if last_mm[0] is not None:
                add_dep_helper(inst, last_mm[0], reason="pe-order")
            last_mm[0] = inst
            return inst

        supers = {}
        gstate = {}

        def gidx(i):
            b = 0
            while i >= GOFF[b] + GRPS[b]:
                b += 1
            r = i - GOFF[b]
            return b, r // SB, r % SB

        def ensure_super(b, s):
            if (b, s) in supers:
                return supers[(b, s)]
            featd = featd_tiles[b]
            featfm = fmp.tile([P, SB * T], BF16, tag="ffm", name=f"ffm{b}_{s}")
            for g in range(4):
                nc.sync.dma_start(
                    out=featfm[32 * g : 32 * g + 6, :],
                    in_=featd[6 * g : 6 * g + 6, SB * T * s : SB * T * (s + 1)],
                )
            x_sb = xp.tile([P, SB * T], FP32, tag="xsb", name=f"xsb{b}_{s}")
            sup = {"ffm_gv": featfm.rearrange("(g r) c -> g r c", g=4), "x_sb": x_sb}
            supers[(b, s)] = sup
            return sup

        def phase_a(i):
            b, s, i2 = gidx(i)
            sup = ensure_super(b, s)
            z1p = [
                psz1.tile([P, 2 * T], FP32, tag="z1", name=f"z1_{i}_0"),
                psz1.tile([P, 2 * T], FP32, tag="z1", name=f"z1_{i}_1"),
            ]
            for g in range(4):
                mm(
                    out=z1p[g // 2][:, (g % 2) * T : (g % 2 + 1) * T],
                    lhsT=w1s_sb[32 * g : 32 * g + 6, :],
                    rhs=sup["ffm_gv"][g, :6, i2 * T : (i2 + 1) * T],
                    tile_position=(32 * g, 0),
                )
            h1p = [
                hp.tile([P, 2 * T], BF16, tag="h1", name=f"h1_{i}_0"),
                hp.tile([P, 2 * T], BF16, tag="h1", name=f"h1_{i}_1"),
            ]
            nc.scalar.activation(
                out=h1p[0][:], in_=z1p[0][:], func=AF.Relu, bias=b1_sb[:]
            )
            gstate[i] = {"h1p": h1p, "z1b": z1p[1]}
            if i2 == SB - 1 and i + 1 < NGROUPS:
                nb, ns, _ = gidx(i + 1)
                if featd_tiles[nb] is not None:
                    ensure_super(nb, ns)

        def relu1_dve(i):
            st = gstate[i]
            nc.vector.tensor_scalar(
                out=st["h1p"][1][:], in0=st.pop("z1b")[:], scalar1=b1_sb[:],
                scalar2=0.0, op0=OP.add, op1=OP.max,
            )

        def phase_b(i):
            st = gstate[i]
            h1p = st["h1p"]
            z2s = [
                psz2.tile([P, T], FP32, tag="z2", name=f"z2_{i}_{g}")
                for g in range(4)
            ]
            for g in range(4):
                mm(
                    out=z2s[g][:],
                    lhsT=w2_sb[:],
                    rhs=h1p[g // 2][:, (g % 2) * T : (g % 2 + 1) * T],
                )
            h2s = []
            for g in range(4):
                h2 = hp.tile([P, T], BF16, tag="h2", name=f"h2_{i}_{g}", bufs=8)
                h2s.append(h2)
                if g % 2 == 0:
                    nc.vector.tensor_scalar(
                        out=h2[:], in0=z2s[g][:], scalar1=b2_sb[:],
                        scalar2=0.0, op0=OP.add, op1=OP.max,
                    )
                else:
                    nc.scalar.activation(
                        out=h2[:], in_=z2s[g][:], func=AF.Relu, bias=b2_sb[:]
                    )
            st["h2s"] = h2s

        def phase_c(i):
            b, s, i2 = gidx(i)
            st = gstate.pop(i)
            sup = supers[(b, s)]
            x_ps = psx.tile([P, T], FP32, tag="x", name=f"xps{i}")
            for g in range(4):
                mm(
                    out=x_ps[32 * g : 32 * g + 32, :],
                    lhsT=w3s_sb[:, :],
                    rhs=st["h2s"][g][:],
                    tile_position=(0, 32 * g),
                )
            nc.scalar.activation(
                out=sup["x_sb"][:, i2 * T : (i2 + 1) * T], in_=x_ps[:],
                func=AF.Identity, bias=b3s_sb[:],
            )
            if i2 == SB - 1:
                xd = xd_tiles[b]
                for g in range(4):
                    nc.sync.dma_start(
                        out=xd[3 * g : 3 * g + 3, SB * T * s : SB * T * (s + 1)],
                        in_=sup["x_sb"][32 * g : 32 * g + 3, :],
                    )
                del supers[(b, s)]

        # ============ stage 3: particle-major backend ============
        def stage3(b):
            tail = b == NBLK - 1
            G = nc.gpsimd
            E = nc.vector if tail else nc.gpsimd
            CB = CBS[b]
            xd = xd_tiles[b]
            f_sb = f_tiles[b]
            pqv = pq_tiles[b].rearrange("p (s c) -> p s c", s=2)
            fr = f_sb.rearrange("p (c k) -> p c k", k=4)

            xs_all = xp.tile([P, 3 * CB], FP32, tag="xsall", name=f"xsall{b}",
                             padded_shape=[P, 3 * max(CBS)])
            for g in range(4):
                nc.sync.dma_start(
                    out=xs_all[32 * g : 32 * g + 32, :].rearrange("i (k j) -> i k j", j=CB),
                    in_=xd[3 * g : 3 * g + 3, :].rearrange("k (i j) -> i k j", j=CB),
                )
            pall = scr.tile([P, 3 * CB], FP32, tag="pall", name=f"pall{b}",
                            padded_shape=[P, 3 * max(CBS)])
            pall_v = pall.rearrange("p (k c) -> p k c", k=3)
            G.tensor_tensor(
                out=pall[:], in0=xs_all[:],
                in1=pqv[:, 0, :].unsqueeze(1).to_broadcast([P, 3, CB]), op=OP.mult,
            )
            qall = scr.tile([P, 3 * CB], FP32, tag="qall", name=f"qall{b}",
                            padded_shape=[P, 3 * max(CBS)])
            qall_v = qall.rearrange("p (k c) -> p k c", k=3)
            E.tensor_tensor(
                out=qall[:], in0=xs_all[:],
                in1=pqv[:, 1, :].unsqueeze(1).to_broadcast([P, 3, CB]), op=OP.mult,
            )
            out_sb = outp.tile([P, 4 * CB], FP32, tag="out", name=f"out_sb{b}",
                               padded_shape=[P, 4 * max(CBS)])
            ov = out_sb.rearrange("p (c k) -> p c k", k=4)
            t0 = scr.tile([P, CB], FP32, tag="t0", name=f"t0_{b}",
                          padded_shape=[P, max(CBS)])
            G.tensor_tensor(out=t0[:], in0=pall_v[:, 0], in1=qall_v[:, 1], op=OP.subtract)
            G.tensor_tensor(out=ov[:, :, 0], in0=t0[:], in1=fr[:, :, 0], op=OP.add)
            t1 = scr.tile([P, CB], FP32, tag="t1", name=f"t1_{b}",
                          padded_shape=[P, max(CBS)])
            E.tensor_tensor(out=t1[:], in0=pall_v[:, 1], in1=qall_v[:, 2], op=OP.subtract)
            E.tensor_tensor(out=ov[:, :, 1], in0=t1[:], in1=fr[:, :, 1], op=OP.add)
            t2 = scr.tile([P, CB], FP32, tag="t2", name=f"t2_{b}",
                          padded_shape=[P, max(CBS)])
            G.tensor_tensor(out=t2[:], in0=qall_v[:, 0], in1=pall_v[:, 1], op=OP.add)
            G.tensor_tensor(out=ov[:, :, 2], in0=t2[:], in1=fr[:, :, 2], op=OP.add)
            t3 = scr.tile([P, CB], FP32, tag="t3", name=f"t3_{b}",
                          padded_shape=[P, max(CBS)])
            E.tensor_tensor(out=t3[:], in0=qall_v[:, 1], in1=pall_v[:, 2], op=OP.add)
            E.tensor_tensor(out=ov[:, :, 3], in0=t3[:], in1=fr[:, :, 3], op=OP.add)

            OUT_bv = OUT[BOFF[b] : BOFF[b] + BLKS[b], :].rearrange(
                "(i g j) k -> i g (j k)", i=32, g=4
            )
            for g in range(4):
                nc.sync.dma_start(out=OUT_bv[:, g, :], in_=out_sb[32 * g : 32 * g + 32, :])

        # ============ emission order (pipelined) ============
        # stage-1 pieces are spread over iterations so cross-engine waits
        # never sit at the head of a busy FIFO.
        front_at, sqrt_at, fin_at = {}, {2: 1}, {3: 1}
        for b in range(2, NBLK):
            ft = GOFF[b] - GRPS[b - 1]
            front_at[ft] = b
            sqrt_at[ft + 7] = b
            fin_at[ft + 9] = b

        stage1_front(0)
        stage1_sqrt(0)
        stage1_finish(0)
        stage1_front(1)
        for t in range(NGROUPS + 2):
            if t < NGROUPS:
                phase_a(t)
            if 0 <= t - 1 < NGROUPS:
                phase_b(t - 1)
            if t < NGROUPS:
                relu1_dve(t)
                if t in front_at:
                    stage1_front(front_at[t])
            if t in sqrt_at:
                stage1_sqrt(sqrt_at[t])
            if t in fin_at:
                stage1_finish(fin_at[t])
            if 0 <= t - 2 < NGROUPS:
                phase_c(t - 2)
                b, s, i2 = gidx(t - 2)
                if s == NSUPS[b] - 1 and i2 == SB - 1:
                    stage3(b)

    nc.finalize()
    return nc


def prep_weights(W1, b1, W2, b2, W3, b3):
    """Host-side weight transforms (tiny)."""
    W1 = np.asarray(W1, np.float32)
    b1 = np.asarray(b1, np.float32)
    W2 = np.asarray(W2, np.float32)
    b2 = np.asarray(b2, np.float32)
    W3 = np.asarray(W3, np.float32)
    b3 = np.asarray(b3, np.float32)
    # features: [sq1, sq2, a2+c2, b2+d2, ab+cd, ad-bc]
    W1eff = np.stack(
        [
            0.5 * (W1[0] + W1[1]),
            0.5 * (W1[0] - W1[1]),
            W1[2],
            W1[5],
            W1[3] + W1[4],
            W1[6],
        ],
        axis=0,
    )  # [6, 128]
    b1eff = b1 - (W1[0] + W1[1] + W1[2] + W1[5] + W1[6])
    W1S = np.zeros((P, P), np.float32)
    for g in range(4):
        W1S[32 * g : 32 * g + 6, :] = W1eff
    # symmetrized third layer: x_sym = [x00, (x01+x10)/2, x11]
    W3S = np.zeros((P, 32), np.float32)
    W3S[:, 0] = W3[:, 0]
    W3S[:, 1] = 0.5 * (W3[:, 1] + W3[:, 2])
    W3S[:, 2] = W3[:, 3]
    b3S3 = np.array([b3[0], 0.5 * (b3[1] + b3[2]), b3[3]], np.float32)
    B3S = np.zeros((P, 1), np.float32)
    for j in range(4):
        B3S[32 * j : 32 * j + 3, 0] = b3S3
    import ml_dtypes
    return {
        "W1S": W1S.astype(ml_dtypes.bfloat16),
        "W2": W2.astype(ml_dtypes.bfloat16),
        "W3S": W3S.astype(ml_dtypes.bfloat16),
        "B1": b1eff.reshape(P, 1).astype(np.float32),
        "B2": b2.reshape(P, 1).astype(np.float32),
        "B3S": B3S,
    }


def kernel(F, W1, b1, W2, b2, W3, b3):
    global _last_results
    F = np.asarray(F, np.float32).reshape(-1, 4)
    n = F.shape[0]
    assert n == N, f"expected {N} particles, got {n}"

    if "nc" not in _built:
        _built["nc"] = build_program()
    nc = _built["nc"]

    wmaps = prep_weights(W1, b1, W2, b2, W3, b3)
    Fpad = np.empty((NTOT, 4), np.float32)
    Fpad[:n] = F
    Fpad[n:] = np.array([1.0, 0.1, 0.0, 1.0], np.float32)

    in_maps = []
    for i in range(NCORES):
        m = {"F": np.ascontiguousarray(Fpad[i * NPC : (i + 1) * NPC])}
        m.update(wmaps)
        in_maps.append(m)

    res = run_bass_kernel_spmd(nc, in_maps, core_ids=list(range(NCORES)))
    _last_results = res
    out = np.concatenate([r["OUT"] for r in res.results], axis=0)[:n]
    return out.reshape(n, 2, 2).astype(np.float32)
